# revision 52
# baseline (speedup 1.0000x reference)
"""Trainium2 Bass kernel for nn_BezierGlyph (SIZE=512, 8 strokes x 32 samples).

Math: out = sigmoid(200*(m - 0.04)), m = -ln(S)/256, S = sum_j exp(-256*d_j)
over the 256 curve samples.  Sharpness 256 makes far samples negligible, so
each 128-pixel tile (16x8 block) only processes its K nearest samples,
selected on the host by tile-centroid distance.  Tiles are split into two
classes and balanced across cores: the 512 densest tiles keep K=48, the
rest keep K=16 (measured max output error of this pruning vs the exact
reference: 8.8e-3, vs the 2e-2 gate).

Device layout (per core: 64 B-slots = K48, 192 A-slots = K16):
  - PE: 4 tiles stack into one matmul group: stationary [72, 128] holds the
    4 tiles' 18 quadratic-form rows (columns = each tile's 128 pixels); the
    moving operand is block-diagonal [72, 4*K] (tile k's point table at
    rows 18k / cols K*k), so one matmul emits 4 tiles' d^2 [128, 4*K].
    16 B-groups (megas 0-1) + 48 A-groups (megas 2-7), PSUM double-buffered.
  - ACT: per-mega Sqrt(PSUM) -> D, then one table switch, per-mega
    Exp(-256 d) -> E, then the epilogue sigmoid(z) = exp(-ln(1+exp(-z)))
    entirely in the natural_log_exp table set; ACT also issues the output
    DMA (HWDGE).  DVE does the segmented per-tile sums.
  - Input DMA: lt on the ACT HWDGE ring, rh on the SP ring, both packed
    over 72 partitions, chunked per mega pair so the PE starts early.
Host: builds tables + pixel permutation; un-permutes the output.  All
input-value-dependent data is DATA (dram params); the BIR is shape-static.

Measured on 8 axon trn2 cores: HW exec ~37.2 us (baseline 164.6 us, 4.4x),
absmax error vs the f32 jax reference 8.03e-3 (deterministic; dominated by
the K-nearest-by-centroid pruning, not device numerics).  The first NEFF
execution after load can race the input upload (pre-existing runtime
hazard, also seen with the baseline kernel) -- _run retries on NaN.
"""
import numpy as np

SIZE = 512
HW = SIZE * SIZE
N_CORES = 8
PXC = HW // N_CORES          # 32768 pixels per core
NT = PXC // 128              # 256 tile-slots per core
TW, TH = 16, 8               # tile geometry (x by y)
NTX, NTY = SIZE // TW, SIZE // TH
KB, KA = 48, 16              # points kept per B/A tile
NB, NA = 64, 192             # B/A slots per core (B = slots 0..63)
NGRP = 4                     # tiles stacked per matmul group
NGB, NGA = NB // NGRP, NA // NGRP      # 16 B-groups, 48 A-groups
GWB, GWA = NGRP * KB, NGRP * KA        # 192 / 64 psum cols per group
NMEGA = 8                    # megas: 0-1 are B (8 groups), 2-7 are A
GPM = 8                      # groups per mega
TPM = 32                     # tiles per mega
RH_COLS = NGB * GWB + NGA * GWA        # 6144
CUTOFF = 0.138               # classification radius (tile centroid)
SHARP = 256.0
GUARD = np.float32(5e-6)
U_SCALE = 200.0 / 256.0
U_BIAS = 8.0 + 2500.0 * float(GUARD)

_CACHE = {}


def _build():
    import concourse.bass as bass
    import concourse.mybir as mybir

    nc = bass.Bass()
    f32 = mybir.dt.float32
    bf16 = mybir.dt.bfloat16
    AF = mybir.ActivationFunctionType

    lt = nc.declare_dram_parameter("lt", [72, (NGB + NGA) * 128], bf16,
                                   isOutput=False)
    rh = nc.declare_dram_parameter("rh", [72, RH_COLS], bf16, isOutput=False)
    out_d = nc.declare_dram_parameter("out", [128, NT], f32, isOutput=True)

    from contextlib import ExitStack
    with ExitStack() as ctx:
        e = ctx.enter_context
        LT = e(nc.sbuf_tensor([72, (NGB + NGA) * 128], bf16))
        RH = e(nc.sbuf_tensor([72, RH_COLS], bf16))
        DB = e(nc.sbuf_tensor([128, 2, TPM, KB], f32))
        DA = e(nc.sbuf_tensor([128, 6, TPM, KA], f32))
        EB = e(nc.sbuf_tensor([128, 2, TPM, KB], f32))
        EA = e(nc.sbuf_tensor([128, 6, TPM, KA], f32))
        SS = e(nc.sbuf_tensor([128, NT], f32))
        LNS = e(nc.sbuf_tensor([128, NT], f32))
        U = e(nc.sbuf_tensor([128, NT], f32))
        R = e(nc.sbuf_tensor([128, NT], f32))
        OUT = e(nc.sbuf_tensor([128, NT], f32))
        WARM = e(nc.sbuf_tensor([128, 1], f32))
        B_LN = e(nc.sbuf_tensor([128, 1], f32))
        B_UB = e(nc.sbuf_tensor([128, 1], f32))
        PA = e(nc.psum_tensor([128, GPM, GWB], f32))
        PB = e(nc.psum_tensor([128, GPM, GWB], f32))
        PC = e(nc.psum_tensor([128, GPM, GWA], f32))
        PD = e(nc.psum_tensor([128, GPM, GWA], f32))
        dma_sem = e(nc.semaphore("dma_sem"))
        rh_sem = e(nc.semaphore("rh_sem"))
        lt_sems = [e(nc.semaphore(f"lt_sem{c}")) for c in range(5)]
        init_sem = e(nc.semaphore("init_sem"))
        mm_sem = e(nc.semaphore("mm_sem"))
        sqrt_sem = e(nc.semaphore("sqrt_sem"))
        exp_sem = e(nc.semaphore("exp_sem"))
        red_sem = e(nc.semaphore("red_sem"))
        fin_sem = e(nc.semaphore("fin_sem"))
        block = e(nc.Block())

        def psum_of(M):
            # B-megas 0-1 double-buffer PA/PB; A-megas rotate 4-deep over
            # PC, PD and the (by then free) 64-col slices of PA/PB so the
            # PE can run ahead and stay busy.
            if M < 2:
                return PSUM_B[M]
            return PSUM_A[(M - 2) % 4]

        PSUM_B = (PA, PB)
        PSUM_A = (PC, PD, PA, PB)
        ONE_AP = nc.const_aps.tensor(1.0, (128, 1))

        def rh_off(gr):
            return GWB * gr if gr < NGB else NGB * GWB + GWA * (gr - NGB)

        def gw_of(M):
            return GWB if M < 2 else GWA

        # rh chunk c covers megas: {0}, {1}, {2,3,4}, {5,6,7}
        RH_CUT = [0, GPM * GWB, 2 * GPM * GWB,
                  2 * GPM * GWB + 3 * GPM * GWA, RH_COLS]
        RH_WAIT = [16, 32, 48, 48, 48, 64, 64, 64]
        # lt chunks of [1,1,2,2,2] megas: the first two are small so the
        # PE's first megas start as early as possible
        LTM = GPM * 128                # lt columns per mega
        LT_CUT = [0, LTM, 2 * LTM, 4 * LTM, 6 * LTM, 8 * LTM]
        LT_OF_M = [0, 1, 2, 2, 3, 3, 4, 4]

        @block.sync
        def _(sp):
            for c in range(4):
                sp.dma_start(RH[:, RH_CUT[c]:RH_CUT[c + 1]],
                             rh[:, RH_CUT[c]:RH_CUT[c + 1]]
                             ).then_inc(rh_sem, 16)

        @block.gpsimd
        def _(g):
            g.memset(B_LN[:, :], 1e-30)
            g.memset(B_UB[:, :], U_BIAS).then_inc(init_sem, 1)

        @block.tensor
        def _(t):
            for M in range(NMEGA):
                if M >= 4:
                    t.wait_ge(sqrt_sem, M - 3)          # psum buffer free
                t.wait_ge(rh_sem, RH_WAIT[M])
                if M == 0 or LT_OF_M[M] != LT_OF_M[M - 1]:
                    t.wait_ge(lt_sems[LT_OF_M[M]], 16)
                P = psum_of(M)
                gw = gw_of(M)
                for j in range(GPM):
                    gr = GPM * M + j                    # group id
                    t.matmul(P[:, j, 0:gw],
                             LT[0:72, 128 * gr: 128 * (gr + 1)],
                             RH[0:72, rh_off(gr): rh_off(gr) + gw],
                             start=True, stop=True, tile_position=(0, 0)
                             ).then_inc(mm_sem, 1)

        @block.scalar
        def _(s):
            # lt input DMAs ride the ACT HWDGE ring (SP ring carries rh)
            for c in range(5):
                nc.scalar.dma_start(LT[:, LT_CUT[c]:LT_CUT[c + 1]],
                                    lt[:, LT_CUT[c]:LT_CUT[c + 1]]
                                    ).then_inc(lt_sems[c], 16)
            # warm the sqrt table while DMAs are in flight
            nc.scalar.activation(WARM[:, :], ONE_AP, AF.Sqrt)
            for M in range(NMEGA):
                s.wait_ge(mm_sem, GPM * (M + 1))
                dst = DB[:, M] if M < 2 else DA[:, M - 2]
                nc.scalar.activation(dst, psum_of(M)[:, :, 0:gw_of(M)],
                                     AF.Sqrt).then_inc(sqrt_sem, 1)
            for M in range(NMEGA):
                src = DB[:, M] if M < 2 else DA[:, M - 2]
                dst = EB[:, M] if M < 2 else EA[:, M - 2]
                nc.scalar.activation(dst, src, AF.Exp,
                                     scale=-SHARP).then_inc(exp_sem, 1)
            s.wait_ge(init_sem, 1)
            # sigmoid(z) = exp(-ln(1 + exp(-z))) -- stays in natural_log_exp
            # set; done in halves so half 0 overlaps the reduce tail
            for h in range(2):
                s.wait_ge(red_sem, 4 * (h + 1))
                cs = slice(128 * h, 128 * (h + 1))
                nc.scalar.activation(LNS[:, cs], SS[:, cs], AF.Ln,
                                     bias=B_LN[:, :])
                nc.scalar.activation(U[:, cs], LNS[:, cs], AF.Exp,
                                     scale=U_SCALE, bias=B_UB[:, :])
                nc.scalar.activation(R[:, cs], U[:, cs], AF.Ln, bias=1.0)
                nc.scalar.activation(OUT[:, cs], R[:, cs], AF.Exp, scale=-1.0
                                     ).then_inc(fin_sem, 1)
                # the HWDGE dispatch runs on the sequencer, which is ahead of
                # the engine pipeline -- gate it on this half's OUT write
                s.wait_ge(fin_sem, h + 1)
                nc.scalar.dma_start(out_d[:, cs], OUT[:, cs]
                                    ).then_inc(dma_sem, 16)
            s.wait_ge(dma_sem, 32)

        @block.vector
        def _(v):
            for M in range(NMEGA):
                v.wait_ge(exp_sem, M + 1)
                src = EB[:, M] if M < 2 else EA[:, M - 2]
                nc.vector.tensor_reduce(SS[:, TPM * M: TPM * (M + 1)], src,
                                        axis=mybir.AxisListType.X,
                                        op=mybir.AluOpType.add
                                        ).then_inc(red_sem, 1)

    return nc


def _bezier_samples(control_points: np.ndarray) -> np.ndarray:
    """(8,4,2) -> (256,2) f32, mirrors the reference's f32 math."""
    pts = np.clip(control_points.astype(np.float32), np.float32(0.0), np.float32(1.0))
    ts = np.linspace(0.0, 1.0, 32).astype(np.float32)
    t = ts[None, :, None]
    mt = np.float32(1.0) - t
    p0, p1, p2, p3 = (pts[:, k: k + 1, :] for k in range(4))
    sam = (mt ** 3 * p0 + np.float32(3.0) * mt ** 2 * t * p1
           + np.float32(3.0) * mt * t ** 2 * p2 + t ** 3 * p3)
    return sam.reshape(-1, 2).astype(np.float32)


def _split_bf3(v64):
    """v (f64) -> 3 bf16 terms summing to v within ~2^-27."""
    import ml_dtypes
    v = np.asarray(v64, np.float64)
    b0 = v.astype(ml_dtypes.bfloat16)
    r = v - b0.astype(np.float64)
    b1 = r.astype(ml_dtypes.bfloat16)
    r2 = r - b1.astype(np.float64)
    b2 = r2.astype(ml_dtypes.bfloat16)
    return b0, b1, b2


def _tile_pixel_ids():
    """(2048, 128) pixel ids: tile g, lane l -> flat pixel index."""
    g = np.arange(NTX * NTY)
    ty, tx = g // NTX, g % NTX
    l = np.arange(128)
    dy, dx = l // TW, l % TW
    y = ty[:, None] * TH + dy[None, :]
    x = tx[:, None] * TW + dx[None, :]
    return (y * SIZE + x).astype(np.int64)


def _point_rows(sam, keep):
    """keep (n, K) sample indices -> (18, n, K) bf16-split rh rows (f64)."""
    sx = (sam[keep, 0] - np.float32(0.5)).astype(np.float64)
    sy = (sam[keep, 1] - np.float32(0.5)).astype(np.float64)
    ah, am, al = _split_bf3(-2.0 * sx)
    bh, bm, bl = _split_bf3(-2.0 * sy)
    s2h, s2m, s2l = _split_bf3(sx * sx + sy * sy + float(GUARD))
    ones = np.ones_like(ah)
    return np.stack([ah, am, al, ah, am, ah,
                     bh, bm, bl, bh, bm, bh,
                     s2h, s2m, s2l, ones, ones, ones])


def _prep_inputs(control_points: np.ndarray, pixel_grid: np.ndarray):
    import ml_dtypes
    sam = _bezier_samples(np.asarray(control_points))        # (256, 2)
    pg = np.asarray(pixel_grid, dtype=np.float32)
    idx0 = _tile_pixel_ids()                                 # (2048, 128)

    # classify tiles by sample count near the centroid; balance across cores
    cx = pg[idx0, 0].mean(axis=1)
    cy = pg[idx0, 1].mean(axis=1)
    dc = np.hypot(cx[:, None] - sam[None, :, 0], cy[:, None] - sam[None, :, 1])
    order = np.argsort(-(dc <= CUTOFF).sum(axis=1), kind="stable")
    Bg, Ag = order[:N_CORES * NB], order[N_CORES * NB:]
    tile_ids = np.concatenate(
        [np.concatenate([Bg[c::N_CORES], Ag[c::N_CORES]])
         for c in range(N_CORES)])                           # slot-ordered
    idx = idx0[tile_ids]                                     # (2048, 128)

    x = (pg[idx, 0] - np.float32(0.5)).astype(np.float64)    # (2048, 128)
    y = (pg[idx, 1] - np.float32(0.5)).astype(np.float64)
    xh, xm, xl = _split_bf3(x)
    yh, ym, yl = _split_bf3(y)
    qh, qm, ql = _split_bf3(x * x + y * y)
    one = np.ones_like(xh)
    lt_rows = np.stack([xh, xh, xh, xm, xm, xl,
                        yh, yh, yh, ym, ym, yl,
                        one, one, one, qh, qm, ql])          # (18, 2048, 128)

    in_maps = []
    for c in range(N_CORES):
        sl = np.arange(c * NT, (c + 1) * NT)
        tid = tile_ids[sl]
        ltc = lt_rows[:, sl].reshape(18, NT // NGRP, NGRP, 128)
        ltv = np.ascontiguousarray(
            ltc.transpose(2, 0, 1, 3).reshape(72, -1)).astype(ml_dtypes.bfloat16)

        rhv = np.zeros((72, RH_COLS), dtype=ml_dtypes.bfloat16)
        kB = np.argpartition(dc[tid[:NB]], KB - 1, axis=1)[:, :KB]
        kA = np.argpartition(dc[tid[NB:]], KA - 1, axis=1)[:, :KA]
        rB = _point_rows(sam, kB).reshape(18, NGB, NGRP, KB)
        rA = _point_rows(sam, kA).reshape(18, NGA, NGRP, KA)
        vB = rhv[:, :NGB * GWB].reshape(72, NGB, NGRP, KB)
        vA = rhv[:, NGB * GWB:].reshape(72, NGA, NGRP, KA)
        for k in range(NGRP):
            vB[18 * k: 18 * (k + 1), :, k, :] = \
                rB[:, :, k, :].astype(ml_dtypes.bfloat16)
            vA[18 * k: 18 * (k + 1), :, k, :] = \
                rA[:, :, k, :].astype(ml_dtypes.bfloat16)
        in_maps.append({"lt": ltv, "rh": np.ascontiguousarray(rhv)})
    return in_maps, idx


def _run(inputs, trace=False):
    from concourse.bass_utils import run_bass_kernel_spmd

    if "nc" not in _CACHE:
        _CACHE["nc"] = _build()
    nc = _CACHE["nc"]
    in_maps, idx = _prep_inputs(inputs["control_points"], inputs["pixel_grid"])
    # The very first NEFF execution after load can race the input upload
    # (observed on the baseline kernel too: sporadic NaN tiles).  NaN is
    # never legitimate here (out = exp(-ln1p(u)) of finite u), so retry.
    for _attempt in range(3):
        res = run_bass_kernel_spmd(nc, in_maps, core_ids=list(range(N_CORES)),
                                   trace=trace)
        outs = [np.asarray(res.results[c]["out"], dtype=np.float32)
                for c in range(N_CORES)]
        if not any(np.isnan(o).any() for o in outs):
            break
    flat = np.empty(HW, dtype=np.float32)
    for c in range(N_CORES):
        flat[idx[c * NT:(c + 1) * NT]] = outs[c].T   # (128, NT): [lane, slot]
    return flat.reshape(1, SIZE, SIZE), res


def kernel(control_points: np.ndarray, pixel_grid: np.ndarray) -> np.ndarray:
    out, _ = _run({"control_points": control_points, "pixel_grid": pixel_grid})
    return out


# revision 54
# speedup vs baseline: 1.0425x; 1.0425x over previous
"""Trainium2 Bass kernel for nn_BezierGlyph (SIZE=512, 8 strokes x 32 samples).

Math: out = sigmoid(200*(m - 0.04)), m = -ln(S)/256, S = sum_j exp(-256*d_j)
over the 256 curve samples.  Sharpness 256 makes far samples negligible, so
each 128-pixel tile (16x8 block) only processes its K nearest samples,
selected on the host by tile-centroid distance.  Tiles are split into two
classes and balanced across cores: the 512 densest tiles keep K=48, the
rest keep K=16 (measured max output error of this pruning vs the exact
reference: 8.8e-3, vs the 2e-2 gate).

Device layout (per core: 64 B-slots = K48, 192 A-slots = K16):
  - PE: 4 tiles stack into one matmul group: stationary [72, 128] holds the
    4 tiles' 18 quadratic-form rows (columns = each tile's 128 pixels); the
    moving operand is block-diagonal [72, 4*K] (tile k's point table at
    rows 18k / cols K*k), so one matmul emits 4 tiles' d^2 [128, 4*K].
    16 B-groups (megas 0-1) + 48 A-groups (megas 2-7), PSUM double-buffered.
  - ACT: per-mega Sqrt(PSUM) -> D, then one table switch, per-mega
    Exp(-256 d) -> E, then the epilogue sigmoid(z) = exp(-ln(1+exp(-z)))
    entirely in the natural_log_exp table set; ACT also issues the output
    DMA (HWDGE).  DVE does the segmented per-tile sums.
  - Input DMA: lt on the ACT HWDGE ring, rh on the SP ring, both packed
    over 72 partitions, chunked per mega pair so the PE starts early.
Host: builds tables + pixel permutation; un-permutes the output.  All
input-value-dependent data is DATA (dram params); the BIR is shape-static.

Measured on 8 axon trn2 cores: HW exec ~37.2 us (baseline 164.6 us, 4.4x),
absmax error vs the f32 jax reference 8.03e-3 (deterministic; dominated by
the K-nearest-by-centroid pruning, not device numerics).  The first NEFF
execution after load can race the input upload (pre-existing runtime
hazard, also seen with the baseline kernel) -- _run retries on NaN.
"""
import numpy as np

SIZE = 512
HW = SIZE * SIZE
N_CORES = 8
PXC = HW // N_CORES          # 32768 pixels per core
NT = PXC // 128              # 256 tile-slots per core
TW, TH = 16, 8               # tile geometry (x by y)
NTX, NTY = SIZE // TW, SIZE // TH
KB, KA = 48, 16              # points kept per B/A tile
NB, NA = 64, 192             # B/A slots per core (B = slots 0..63)
NGRP = 4                     # tiles stacked per matmul group
NGB, NGA = NB // NGRP, NA // NGRP      # 16 B-groups, 48 A-groups
GWB, GWA = NGRP * KB, NGRP * KA        # 192 / 64 psum cols per group
NMEGA = 8                    # megas: 0-1 are B (8 groups), 2-7 are A
GPM = 8                      # groups per mega
TPM = 32                     # tiles per mega
RH_COLS = NGB * GWB + NGA * GWA        # 6144
CUTOFF = 0.138               # classification radius (tile centroid)
SHARP = 256.0
GUARD = np.float32(5e-6)
U_SCALE = 200.0 / 256.0
U_BIAS = 8.0 + 2500.0 * float(GUARD)

_CACHE = {}


def _build():
    import concourse.bass as bass
    import concourse.mybir as mybir

    nc = bass.Bass()
    f32 = mybir.dt.float32
    bf16 = mybir.dt.bfloat16
    AF = mybir.ActivationFunctionType

    lt = nc.declare_dram_parameter("lt", [72, (NGB + NGA) * 128], bf16,
                                   isOutput=False)
    rh = nc.declare_dram_parameter("rh", [72, RH_COLS], bf16, isOutput=False)
    out_d = nc.declare_dram_parameter("out", [128, NT], f32, isOutput=True)

    from contextlib import ExitStack
    with ExitStack() as ctx:
        e = ctx.enter_context
        LT = e(nc.sbuf_tensor([72, (NGB + NGA) * 128], bf16))
        RH = e(nc.sbuf_tensor([72, RH_COLS], bf16))
        DB = e(nc.sbuf_tensor([128, 2, TPM, KB], f32))
        DA = e(nc.sbuf_tensor([128, 6, TPM, KA], f32))
        EB = e(nc.sbuf_tensor([128, 2, TPM, KB], f32))
        EA = e(nc.sbuf_tensor([128, 6, TPM, KA], f32))
        SS = e(nc.sbuf_tensor([128, NT], f32))
        LNS = e(nc.sbuf_tensor([128, NT], f32))
        U = e(nc.sbuf_tensor([128, NT], f32))
        R = e(nc.sbuf_tensor([128, NT], f32))
        OUT = e(nc.sbuf_tensor([128, NT], f32))
        WARM = e(nc.sbuf_tensor([128, 1], f32))
        B_LN = e(nc.sbuf_tensor([128, 1], f32))
        B_UB = e(nc.sbuf_tensor([128, 1], f32))
        PA = e(nc.psum_tensor([128, GPM, GWB], f32))
        PB = e(nc.psum_tensor([128, GPM, GWB], f32))
        PC = e(nc.psum_tensor([128, GPM, GWA], f32))
        PD = e(nc.psum_tensor([128, GPM, GWA], f32))
        dma_sem = e(nc.semaphore("dma_sem"))
        rh_sem = e(nc.semaphore("rh_sem"))
        lt_sems = [e(nc.semaphore(f"lt_sem{c}")) for c in range(5)]
        init_sem = e(nc.semaphore("init_sem"))
        mm_sem = e(nc.semaphore("mm_sem"))
        sqrt_sem = e(nc.semaphore("sqrt_sem"))
        exp_sem = e(nc.semaphore("exp_sem"))
        red_sem = e(nc.semaphore("red_sem"))
        fin_sem = e(nc.semaphore("fin_sem"))
        block = e(nc.Block())

        def psum_of(M):
            # B-megas 0-1 double-buffer PA/PB; A-megas rotate 4-deep over
            # PC, PD and the (by then free) 64-col slices of PA/PB so the
            # PE can run ahead and stay busy.
            if M < 2:
                return PSUM_B[M]
            return PSUM_A[(M - 2) % 4]

        PSUM_B = (PA, PB)
        PSUM_A = (PC, PD, PA, PB)
        ONE_AP = nc.const_aps.tensor(1.0, (128, 1))

        def rh_off(gr):
            return GWB * gr if gr < NGB else NGB * GWB + GWA * (gr - NGB)

        def gw_of(M):
            return GWB if M < 2 else GWA

        # rh chunk c covers megas: {0}, {1}, {2,3,4}, {5,6,7}
        RH_CUT = [0, GPM * GWB, 2 * GPM * GWB,
                  2 * GPM * GWB + 3 * GPM * GWA, RH_COLS]
        RH_WAIT = [16, 32, 48, 48, 48, 64, 64, 64]
        # lt quarters of 2 megas each; more/smaller chunks measured slower
        # (each extra dispatch costs ~0.8-1.6us on the ACT sequencer, which
        # delays the warm-up and first sqrt more than the PE gains)
        LTM = GPM * 128                # lt columns per mega
        LT_CUT = [0, 2 * LTM, 4 * LTM, 6 * LTM, 8 * LTM]
        LT_OF_M = [0, 0, 1, 1, 2, 2, 3, 3]

        @block.sync
        def _(sp):
            for c in range(4):
                sp.dma_start(RH[:, RH_CUT[c]:RH_CUT[c + 1]],
                             rh[:, RH_CUT[c]:RH_CUT[c + 1]]
                             ).then_inc(rh_sem, 16)

        @block.gpsimd
        def _(g):
            g.memset(B_LN[:, :], 1e-30)
            g.memset(B_UB[:, :], U_BIAS).then_inc(init_sem, 1)

        @block.tensor
        def _(t):
            for M in range(NMEGA):
                if M >= 4:
                    t.wait_ge(sqrt_sem, M - 3)          # psum buffer free
                t.wait_ge(rh_sem, RH_WAIT[M])
                if M == 0 or LT_OF_M[M] != LT_OF_M[M - 1]:
                    t.wait_ge(lt_sems[LT_OF_M[M]], 16)
                P = psum_of(M)
                gw = gw_of(M)
                for j in range(GPM):
                    gr = GPM * M + j                    # group id
                    t.matmul(P[:, j, 0:gw],
                             LT[0:72, 128 * gr: 128 * (gr + 1)],
                             RH[0:72, rh_off(gr): rh_off(gr) + gw],
                             start=True, stop=True, tile_position=(0, 0)
                             ).then_inc(mm_sem, 1)

        @block.scalar
        def _(s):
            # lt input DMAs ride the ACT HWDGE ring (SP ring carries rh)
            for c in range(4):
                nc.scalar.dma_start(LT[:, LT_CUT[c]:LT_CUT[c + 1]],
                                    lt[:, LT_CUT[c]:LT_CUT[c + 1]]
                                    ).then_inc(lt_sems[c], 16)
            # warm the sqrt table while DMAs are in flight
            nc.scalar.activation(WARM[:, :], ONE_AP, AF.Sqrt)
            for M in range(NMEGA):
                s.wait_ge(mm_sem, GPM * (M + 1))
                dst = DB[:, M] if M < 2 else DA[:, M - 2]
                nc.scalar.activation(dst, psum_of(M)[:, :, 0:gw_of(M)],
                                     AF.Sqrt).then_inc(sqrt_sem, 1)
            for M in range(NMEGA):
                src = DB[:, M] if M < 2 else DA[:, M - 2]
                dst = EB[:, M] if M < 2 else EA[:, M - 2]
                nc.scalar.activation(dst, src, AF.Exp,
                                     scale=-SHARP).then_inc(exp_sem, 1)
            s.wait_ge(init_sem, 1)
            # sigmoid(z) = exp(-ln(1 + exp(-z))) -- stays in natural_log_exp
            # set; done in halves so half 0 overlaps the reduce tail
            for h in range(2):
                s.wait_ge(red_sem, 4 * (h + 1))
                cs = slice(128 * h, 128 * (h + 1))
                nc.scalar.activation(LNS[:, cs], SS[:, cs], AF.Ln,
                                     bias=B_LN[:, :])
                nc.scalar.activation(U[:, cs], LNS[:, cs], AF.Exp,
                                     scale=U_SCALE, bias=B_UB[:, :])
                nc.scalar.activation(R[:, cs], U[:, cs], AF.Ln, bias=1.0)
                nc.scalar.activation(OUT[:, cs], R[:, cs], AF.Exp, scale=-1.0
                                     ).then_inc(fin_sem, 1)
                # the HWDGE dispatch runs on the sequencer, which is ahead of
                # the engine pipeline -- gate it on this half's OUT write
                s.wait_ge(fin_sem, h + 1)
                nc.scalar.dma_start(out_d[:, cs], OUT[:, cs]
                                    ).then_inc(dma_sem, 16)
            s.wait_ge(dma_sem, 32)

        @block.vector
        def _(v):
            for M in range(NMEGA):
                v.wait_ge(exp_sem, M + 1)
                src = EB[:, M] if M < 2 else EA[:, M - 2]
                nc.vector.tensor_reduce(SS[:, TPM * M: TPM * (M + 1)], src,
                                        axis=mybir.AxisListType.X,
                                        op=mybir.AluOpType.add
                                        ).then_inc(red_sem, 1)

    return nc


def _bezier_samples(control_points: np.ndarray) -> np.ndarray:
    """(8,4,2) -> (256,2) f32, mirrors the reference's f32 math."""
    pts = np.clip(control_points.astype(np.float32), np.float32(0.0), np.float32(1.0))
    ts = np.linspace(0.0, 1.0, 32).astype(np.float32)
    t = ts[None, :, None]
    mt = np.float32(1.0) - t
    p0, p1, p2, p3 = (pts[:, k: k + 1, :] for k in range(4))
    sam = (mt ** 3 * p0 + np.float32(3.0) * mt ** 2 * t * p1
           + np.float32(3.0) * mt * t ** 2 * p2 + t ** 3 * p3)
    return sam.reshape(-1, 2).astype(np.float32)


def _split_bf3(v64):
    """v (f64) -> 3 bf16 terms summing to v within ~2^-27."""
    import ml_dtypes
    v = np.asarray(v64, np.float64)
    b0 = v.astype(ml_dtypes.bfloat16)
    r = v - b0.astype(np.float64)
    b1 = r.astype(ml_dtypes.bfloat16)
    r2 = r - b1.astype(np.float64)
    b2 = r2.astype(ml_dtypes.bfloat16)
    return b0, b1, b2


def _tile_pixel_ids():
    """(2048, 128) pixel ids: tile g, lane l -> flat pixel index."""
    g = np.arange(NTX * NTY)
    ty, tx = g // NTX, g % NTX
    l = np.arange(128)
    dy, dx = l // TW, l % TW
    y = ty[:, None] * TH + dy[None, :]
    x = tx[:, None] * TW + dx[None, :]
    return (y * SIZE + x).astype(np.int64)


def _point_rows(sam, keep):
    """keep (n, K) sample indices -> (18, n, K) bf16-split rh rows (f64)."""
    sx = (sam[keep, 0] - np.float32(0.5)).astype(np.float64)
    sy = (sam[keep, 1] - np.float32(0.5)).astype(np.float64)
    ah, am, al = _split_bf3(-2.0 * sx)
    bh, bm, bl = _split_bf3(-2.0 * sy)
    s2h, s2m, s2l = _split_bf3(sx * sx + sy * sy + float(GUARD))
    ones = np.ones_like(ah)
    return np.stack([ah, am, al, ah, am, ah,
                     bh, bm, bl, bh, bm, bh,
                     s2h, s2m, s2l, ones, ones, ones])


def _prep_inputs(control_points: np.ndarray, pixel_grid: np.ndarray):
    import ml_dtypes
    sam = _bezier_samples(np.asarray(control_points))        # (256, 2)
    pg = np.asarray(pixel_grid, dtype=np.float32)
    idx0 = _tile_pixel_ids()                                 # (2048, 128)

    # classify tiles by sample count near the centroid; balance across cores
    cx = pg[idx0, 0].mean(axis=1)
    cy = pg[idx0, 1].mean(axis=1)
    dc = np.hypot(cx[:, None] - sam[None, :, 0], cy[:, None] - sam[None, :, 1])
    order = np.argsort(-(dc <= CUTOFF).sum(axis=1), kind="stable")
    Bg, Ag = order[:N_CORES * NB], order[N_CORES * NB:]
    tile_ids = np.concatenate(
        [np.concatenate([Bg[c::N_CORES], Ag[c::N_CORES]])
         for c in range(N_CORES)])                           # slot-ordered
    idx = idx0[tile_ids]                                     # (2048, 128)

    x = (pg[idx, 0] - np.float32(0.5)).astype(np.float64)    # (2048, 128)
    y = (pg[idx, 1] - np.float32(0.5)).astype(np.float64)
    xh, xm, xl = _split_bf3(x)
    yh, ym, yl = _split_bf3(y)
    qh, qm, ql = _split_bf3(x * x + y * y)
    one = np.ones_like(xh)
    lt_rows = np.stack([xh, xh, xh, xm, xm, xl,
                        yh, yh, yh, ym, ym, yl,
                        one, one, one, qh, qm, ql])          # (18, 2048, 128)

    in_maps = []
    for c in range(N_CORES):
        sl = np.arange(c * NT, (c + 1) * NT)
        tid = tile_ids[sl]
        ltc = lt_rows[:, sl].reshape(18, NT // NGRP, NGRP, 128)
        ltv = np.ascontiguousarray(
            ltc.transpose(2, 0, 1, 3).reshape(72, -1)).astype(ml_dtypes.bfloat16)

        rhv = np.zeros((72, RH_COLS), dtype=ml_dtypes.bfloat16)
        kB = np.argpartition(dc[tid[:NB]], KB - 1, axis=1)[:, :KB]
        kA = np.argpartition(dc[tid[NB:]], KA - 1, axis=1)[:, :KA]
        rB = _point_rows(sam, kB).reshape(18, NGB, NGRP, KB)
        rA = _point_rows(sam, kA).reshape(18, NGA, NGRP, KA)
        vB = rhv[:, :NGB * GWB].reshape(72, NGB, NGRP, KB)
        vA = rhv[:, NGB * GWB:].reshape(72, NGA, NGRP, KA)
        for k in range(NGRP):
            vB[18 * k: 18 * (k + 1), :, k, :] = \
                rB[:, :, k, :].astype(ml_dtypes.bfloat16)
            vA[18 * k: 18 * (k + 1), :, k, :] = \
                rA[:, :, k, :].astype(ml_dtypes.bfloat16)
        in_maps.append({"lt": ltv, "rh": np.ascontiguousarray(rhv)})
    return in_maps, idx


def _run(inputs, trace=False):
    from concourse.bass_utils import run_bass_kernel_spmd

    if "nc" not in _CACHE:
        _CACHE["nc"] = _build()
    nc = _CACHE["nc"]
    in_maps, idx = _prep_inputs(inputs["control_points"], inputs["pixel_grid"])
    # The very first NEFF execution after load can race the input upload
    # (observed on the baseline kernel too: sporadic NaN tiles).  NaN is
    # never legitimate here (out = exp(-ln1p(u)) of finite u), so retry.
    for _attempt in range(3):
        res = run_bass_kernel_spmd(nc, in_maps, core_ids=list(range(N_CORES)),
                                   trace=trace)
        outs = [np.asarray(res.results[c]["out"], dtype=np.float32)
                for c in range(N_CORES)]
        if not any(np.isnan(o).any() for o in outs):
            break
    flat = np.empty(HW, dtype=np.float32)
    for c in range(N_CORES):
        flat[idx[c * NT:(c + 1) * NT]] = outs[c].T   # (128, NT): [lane, slot]
    return flat.reshape(1, SIZE, SIZE), res


def kernel(control_points: np.ndarray, pixel_grid: np.ndarray) -> np.ndarray:
    out, _ = _run({"control_points": control_points, "pixel_grid": pixel_grid})
    return out
